# revision 42
# baseline (speedup 1.0000x reference)
"""Trainium2 Bass kernel for the L1Writer scatter-memory problem.

Computes   out = 0.95 * memory + einsum('bs,bshk,bshv->hkv', rho, keys, values)

Strategy: data-parallel over the flattened (B*S)=16384 token axis, 2048 rows
per core.  Each core computes its partial delta
    delta_h = (sqrt(rho) K_h)^T (sqrt(rho) V_h)     (per head h)
as a chain of 128-row PE matmuls accumulating in PSUM.  The 8 partial
(H,Dk,Dv) deltas are summed on the host (tiny: 256 KB each) and added to
decay*memory there.

sqrt(rho) is folded into BOTH keys and values on the host and the results
are cast to fp8 e4m3 (PSUM accumulation stays fp32; fp8 products are exact
in fp32, so the device result matches a host simulation of the quantized
inputs).  Measured end-to-end rel err 7.2e-3 on the fixed reference inputs,
inside the 2e-2 gate with 2.8x margin.  fp8 quarters HBM traffic vs fp32 --
the kernel is memory-bound -- and enables 128-column weight loads (two
adjacent heads per LDWEIGHTS) to halve the PE's weight-load bandwidth floor.

Per-core kernel layout:
  - keys/values arrive as (128, 16, 1024) fp8, host-transposed so that
    token c*128+p lives at [p, c, :]: every DMA partition line is one
    contiguous run in DRAM.  Pieces are sized small at the start (so the
    PE starts early) and small at the end (so the post-DMA tail is one
    chunk's matmuls); every DMA gets its own semaphore -- a shared
    counting semaphore across DMAs is unsound (16 SDMA engines inc
    independently and drain at different rates).
  - Per chunk c and head pair q: one LDWEIGHTS of K[:, heads 2q,2q+1]
    (128 columns -- FWL-eligible) and one 128-wide matmul against
    V[:, heads 2q,2q+1] accumulating into acc[q//4][:, q%4, :, :]
    ([128, 4, 2, 64] per PSUM bank).  Rows 0-63 of block half 0 hold
    head 2q's delta; rows 64-127 of half 1 hold head 2q+1's; the other
    two quadrants are don't-care cross products.  All matmuls use
    start=False onto DVE-memset zeros (first write per element
    overwrites or accumulates onto zero -- correct for any stale
    has_written bits).
  - The last chunk runs bank-0 pairs first: DVE evacuates bank 0's two
    valid quadrant strips (partition-aligned copies) while PE finishes
    bank 1, ACT DMAs the bank-0 half of the output (its own HWDGE ring)
    while DVE copies bank 1, then sync DMAs the bank-1 half.  DRAM
    delta is (128, 512) fp32 = [k | 64+k][g][q][v]; host reassembles
    (H, Dk, Dv) with h = 2*(4g+q) (+1 for rows 64-127).
"""

import numpy as np
import ml_dtypes

F8 = ml_dtypes.float8_e4m3   # matches mybir.dt.float8e4 on this platform

DECAY = 0.95
B, S, H, Dk, Dv = 4, 4096, 16, 64, 64
N_CORES = 8
NS = (B * S) // N_CORES          # 2048 rows per core
P = 128                          # partitions
CHUNKS = NS // P                 # 16 contraction chunks of 128 rows
FD = H * Dk                      # 1024 features per row
NPAIR = H // 2                   # 8 head pairs per chunk

# (start_chunk, end_chunk) per DMA piece; small at the start for early PE
# start, small at the end for a one-chunk post-DMA tail.  Keys stream on the
# ACT HWDGE ring and values on the SP ring with the same piece boundaries:
# the two descriptor generators run in parallel (one ring can't stay fed --
# generation costs ~0.6-1.1us per trigger) and chunk c's keys and values
# arrive together since both rings drain at the same rate.
K_PIECES = [(0, 2), (2, 6), (6, 10), (10, 13), (13, 15), (15, 16)]
V_PIECES = [(0, 2), (2, 6), (6, 10), (10, 13), (13, 15), (15, 16)]

_nc_cache = None


def _build_nc():
    from contextlib import ExitStack

    import concourse.bass as bass
    import concourse.mybir as mybir

    f32 = mybir.dt.float32
    f8 = mybir.dt.float8e4
    nc = bass.Bass()

    keys_d = nc.dram_tensor("keys", (P, CHUNKS, FD), f8, kind="ExternalInput")
    vals_d = nc.dram_tensor("values", (P, CHUNKS, FD), f8, kind="ExternalInput")
    out_d = nc.dram_tensor("delta", (P, 512), f32, kind="ExternalOutput")

    # chunk -> index of the key/value piece that carries it
    k_of = [next(i for i, (a, b) in enumerate(K_PIECES) if a <= c < b) for c in range(CHUNKS)]
    v_of = [next(i for i, (a, b) in enumerate(V_PIECES) if a <= c < b) for c in range(CHUNKS)]

    # Raw bass (no Tile); all waits are standalone sequencer wait_ge ops.
    #
    # Semaphore budget: ksem[6] + vsem[6] (one per DMA, exact completion) +
    # dve_sem (1 memset + 4 evac copies) + pe_sem (+1 per chunk 0..14, +1
    # after last chunk's bank-0 pairs, +1 after bank-1 pairs) + out_sem
    # (+16 per output half; gpsimd waits 32 before clearing).
    with ExitStack() as ctx:
        kt = ctx.enter_context(nc.sbuf_tensor("kt", [P, CHUNKS, FD], f8))
        vt = ctx.enter_context(nc.sbuf_tensor("vt", [P, CHUNKS, FD], f8))
        # out_big[p, g, q, v]: bank-g pair-q quadrant strips
        out_big = ctx.enter_context(nc.sbuf_tensor("out_big", [P, 2, 4, Dv], f32))
        acc = [
            ctx.enter_context(nc.psum_tensor(f"acc{i}", [P, 4, 2, Dv], f32))
            for i in range(2)
        ]
        ksem = [ctx.enter_context(nc.semaphore(name=f"ks{i}")) for i in range(len(K_PIECES))]
        vsem = [ctx.enter_context(nc.semaphore(name=f"vs{i}")) for i in range(len(V_PIECES))]
        dve_sem = ctx.enter_context(nc.semaphore(name="dve_sem"))
        pe_sem = ctx.enter_context(nc.semaphore(name="pe_sem"))
        out_sem = ctx.enter_context(nc.semaphore(name="out_sem"))
        block = ctx.enter_context(nc.Block())

        @block.sync
        def _(sync):
            for vi, (a, b) in enumerate(V_PIECES):
                sync.dma_start(vt[:, a:b, :], vals_d[:, a:b, :]).then_inc(
                    vsem[vi], 16
                )
            sync.wait_ge(dve_sem, 5)
            sync.dma_start(out_d[:, 256:512], out_big[:, 1, :, :]).then_inc(
                out_sem, 16
            )

        @block.gpsimd
        def _(gpsimd):
            # Semaphores persist across NEFF executions; clear them before
            # the next run.  Input/compute semaphores are provably retired
            # once dve_sem hits 5 (PE consumed every ksem/vsem wait and DVE
            # consumed pe_sem before its last copy), so those clears hide
            # under the output DMAs' HBM write receipt; only out_sem's own
            # clear has to wait for the receipt.
            gpsimd.wait_ge(dve_sem, 5)
            for s in [*ksem, *vsem, pe_sem]:
                gpsimd.sem_clear(s)
            gpsimd.wait_ge(out_sem, 32)
            gpsimd.sem_clear(out_sem)
            gpsimd.sem_clear(dve_sem)

        @block.scalar
        def _(scalar):
            # Keys stream on ACT's HWDGE ring, in parallel with values on SP.
            for ki, (a, b) in enumerate(K_PIECES):
                scalar.dma_start(kt[:, a:b, :], keys_d[:, a:b, :]).then_inc(
                    ksem[ki], 16
                )
            # Then the bank-0 half of the output, while DVE still evacuates
            # bank 1.  ACT does no datapath work (an ACT copy feeding an ACT
            # dma_start raced -- the sequencer-level DMA trigger does not
            # order against the in-flight datapath op).
            scalar.wait_ge(dve_sem, 3)
            scalar.dma_start(out_d[:, 0:256], out_big[:, 0, :, :]).then_inc(
                out_sem, 16
            )

        @block.vector
        def _(vector):
            vector.memset(acc[0][:], 0.0)
            vector.memset(acc[1][:], 0.0).then_inc(dve_sem, 1)
            # bank 0 strips while PE finishes bank 1 (different banks)
            vector.wait_ge(pe_sem, 16)
            vector.tensor_copy(out_big[0:64, 0, :, :], acc[0][0:64, :, 0, :]).then_inc(
                dve_sem, 1
            )
            vector.tensor_copy(
                out_big[64:128, 0, :, :], acc[0][64:128, :, 1, :]
            ).then_inc(dve_sem, 1)
            vector.wait_ge(pe_sem, 17)
            vector.tensor_copy(out_big[0:64, 1, :, :], acc[1][0:64, :, 0, :]).then_inc(
                dve_sem, 1
            )
            vector.tensor_copy(
                out_big[64:128, 1, :, :], acc[1][64:128, :, 1, :]
            ).then_inc(dve_sem, 1)

        @block.tensor
        def _(tensor):
            last_k = last_v = -1
            for c in range(CHUNKS):
                if k_of[c] != last_k:
                    last_k = k_of[c]
                    tensor.wait_ge(ksem[last_k], 16)
                if v_of[c] != last_v:
                    last_v = v_of[c]
                    tensor.wait_ge(vsem[last_v], 16)
                if c == 0:
                    tensor.wait_ge(dve_sem, 1)  # memsets done
                last = c == CHUNKS - 1
                for q in range(NPAIR):
                    g, qq = divmod(q, 4)
                    mm = tensor.matmul(
                        acc[g][:, qq, :, :],
                        kt[:, c, q * 128 : (q + 1) * 128],
                        vt[:, c, q * 128 : (q + 1) * 128],
                        start=False,
                        stop=last and (q == 3 or q == NPAIR - 1),
                        skip_group_check=True,
                    )
                    if (last and q == 3) or q == NPAIR - 1:
                        mm.then_inc(pe_sem, 1)

    return nc


def _get_nc():
    global _nc_cache
    if _nc_cache is None:
        _nc_cache = _build_nc()
    return _nc_cache


def _make_in_maps(keys, values, write_strengths):
    # Fold sqrt(rho) into both operands in fp32, one fp8 cast for the full
    # arrays, then per-core (chunk, p, f) -> (p, chunk, f) transposes so
    # each DMA partition line is contiguous in DRAM.
    sq = np.sqrt(np.asarray(write_strengths, dtype=np.float32)).reshape(B * S, 1)
    kq = (keys.reshape(B * S, FD) * sq).astype(F8)
    vq = (values.reshape(B * S, FD) * sq).astype(F8)
    in_maps = []
    for c in range(N_CORES):
        sl = slice(c * NS, (c + 1) * NS)
        in_maps.append(
            {
                "keys": np.ascontiguousarray(
                    kq[sl].reshape(CHUNKS, P, FD).transpose(1, 0, 2)
                ),
                "values": np.ascontiguousarray(
                    vq[sl].reshape(CHUNKS, P, FD).transpose(1, 0, 2)
                ),
            }
        )
    return in_maps


def _run(in_maps, **kwargs):
    from concourse.bass_utils import run_bass_kernel_spmd

    nc = _get_nc()
    return run_bass_kernel_spmd(nc, in_maps, core_ids=list(range(N_CORES)), **kwargs)


def _assemble(memory, results):
    parts = np.stack([r["delta"] for r in results], axis=0)  # (8, 128, 512)
    arr = parts.sum(axis=0, dtype=np.float64)  # (128, 512)
    blk = arr.reshape(128, 2, 4, Dv)
    delta_hkv = np.empty((H, Dk, Dv))
    for g in range(2):
        for q in range(4):
            delta_hkv[2 * (4 * g + q)] = blk[0:64, g, q, :]
            delta_hkv[2 * (4 * g + q) + 1] = blk[64:128, g, q, :]
    out = DECAY * np.asarray(memory, dtype=np.float64) + delta_hkv
    return out.astype(np.float32)


def kernel(memory, keys, values, write_strengths):
    memory = np.asarray(memory, dtype=np.float32)
    keys = np.asarray(keys, dtype=np.float32)
    values = np.asarray(values, dtype=np.float32)
    write_strengths = np.asarray(write_strengths, dtype=np.float32)

    in_maps = _make_in_maps(keys, values, write_strengths)
    res = _run(in_maps)
    return _assemble(memory, res.results)


if __name__ == "__main__":
    rng = np.random.default_rng(0)
    mem = rng.standard_normal((H, Dk, Dv), dtype=np.float32)
    k = rng.standard_normal((B, S, H, Dk), dtype=np.float32)
    v = rng.standard_normal((B, S, H, Dv), dtype=np.float32)
    w = rng.random((B, S), dtype=np.float32)
    out = kernel(mem, k, v, w)
    ref = DECAY * mem + np.einsum(
        "bs,bshk,bshv->hkv", w.astype(np.float64), k.astype(np.float64), v.astype(np.float64)
    )
    err = np.abs(out - ref).max() / np.abs(ref).max()
    print("self-check rel err:", err)


# revision 46
# speedup vs baseline: 1.0079x; 1.0079x over previous
"""Trainium2 Bass kernel for the L1Writer scatter-memory problem.

Computes   out = 0.95 * memory + einsum('bs,bshk,bshv->hkv', rho, keys, values)

Strategy: data-parallel over the flattened (B*S)=16384 token axis, 2048 rows
per core.  Each core computes its partial delta
    delta_h = (sqrt(rho) K_h)^T (sqrt(rho) V_h)     (per head h)
as a chain of 128-row PE matmuls accumulating in PSUM.  The 8 partial
(H,Dk,Dv) deltas are summed on the host (tiny: 256 KB each) and added to
decay*memory there.

sqrt(rho) is folded into BOTH keys and values on the host and the results
are cast to fp8 e4m3 (PSUM accumulation stays fp32; fp8 products are exact
in fp32, so the device result matches a host simulation of the quantized
inputs).  Measured end-to-end rel err 7.2e-3 on the fixed reference inputs,
inside the 2e-2 gate with 2.8x margin.  fp8 quarters HBM traffic vs fp32 --
the kernel is memory-bound -- and enables 128-column weight loads (two
adjacent heads per LDWEIGHTS) to halve the PE's weight-load bandwidth floor.

Per-core kernel layout:
  - keys/values arrive as (128, 16, 1024) fp8, host-transposed so that
    token c*128+p lives at [p, c, :]: every DMA partition line is one
    contiguous run in DRAM.  Pieces are sized small at the start (so the
    PE starts early) and small at the end (so the post-DMA tail is one
    chunk's matmuls); every DMA gets its own semaphore -- a shared
    counting semaphore across DMAs is unsound (16 SDMA engines inc
    independently and drain at different rates).
  - Per chunk c and head pair q: one LDWEIGHTS of K[:, heads 2q,2q+1]
    (128 columns -- FWL-eligible) and one 128-wide matmul against
    V[:, heads 2q,2q+1] accumulating into acc[q//4][:, q%4, :, :]
    ([128, 4, 2, 64] per PSUM bank).  Rows 0-63 of block half 0 hold
    head 2q's delta; rows 64-127 of half 1 hold head 2q+1's; the other
    two quadrants are don't-care cross products.  All matmuls use
    start=False onto DVE-memset zeros (first write per element
    overwrites or accumulates onto zero -- correct for any stale
    has_written bits).
  - The last chunk runs bank-0 pairs first: DVE evacuates bank 0's two
    valid quadrant strips (partition-aligned copies) while PE finishes
    bank 1, ACT DMAs the bank-0 half of the output (its own HWDGE ring)
    while DVE copies bank 1, then sync DMAs the bank-1 half.  DRAM
    delta is (128, 512) fp32 = [k | 64+k][g][q][v]; host reassembles
    (H, Dk, Dv) with h = 2*(4g+q) (+1 for rows 64-127).
"""

import numpy as np
import ml_dtypes

F8 = ml_dtypes.float8_e4m3   # matches mybir.dt.float8e4 on this platform

DECAY = 0.95
B, S, H, Dk, Dv = 4, 4096, 16, 64, 64
N_CORES = 8
NS = (B * S) // N_CORES          # 2048 rows per core
P = 128                          # partitions
CHUNKS = NS // P                 # 16 contraction chunks of 128 rows
FD = H * Dk                      # 1024 features per row
NPAIR = H // 2                   # 8 head pairs per chunk

# (start_chunk, end_chunk) per DMA piece; small at the start for early PE
# start, small at the end for a one-chunk post-DMA tail.  Keys stream on the
# ACT HWDGE ring and values on the SP ring with the same piece boundaries:
# the two descriptor generators run in parallel (one ring can't stay fed --
# generation costs ~0.6-1.1us per trigger) and chunk c's keys and values
# arrive together since both rings drain at the same rate.
K_PIECES = [(0, 2), (2, 6), (6, 10), (10, 13), (13, 15), (15, 16)]
V_PIECES = [(0, 2), (2, 6), (6, 10), (10, 13), (13, 15), (15, 16)]

_nc_cache = None


def _build_nc():
    from contextlib import ExitStack

    import concourse.bass as bass
    import concourse.mybir as mybir

    f32 = mybir.dt.float32
    f8 = mybir.dt.float8e4
    nc = bass.Bass()

    keys_d = nc.dram_tensor("keys", (P, CHUNKS, FD), f8, kind="ExternalInput")
    vals_d = nc.dram_tensor("values", (P, CHUNKS, FD), f8, kind="ExternalInput")
    out_d = nc.dram_tensor("delta", (P, 512), f32, kind="ExternalOutput")

    # chunk -> index of the key/value piece that carries it
    k_of = [next(i for i, (a, b) in enumerate(K_PIECES) if a <= c < b) for c in range(CHUNKS)]
    v_of = [next(i for i, (a, b) in enumerate(V_PIECES) if a <= c < b) for c in range(CHUNKS)]

    # Raw bass (no Tile); all waits are standalone sequencer wait_ge ops.
    #
    # Semaphore budget: ksem[6] + vsem[6] (one per DMA, exact completion) +
    # dve_sem (1 memset + 4 evac copies) + pe_sem (+1 per chunk 0..14, +1
    # after last chunk's bank-0 pairs, +1 after bank-1 pairs) + out_sem
    # (+16 per output half; sync waits 32) + done_sem.
    with ExitStack() as ctx:
        kt = ctx.enter_context(nc.sbuf_tensor("kt", [P, CHUNKS, FD], f8))
        vt = ctx.enter_context(nc.sbuf_tensor("vt", [P, CHUNKS, FD], f8))
        # out_big[p, g, q, v]: bank-g pair-q quadrant strips
        out_big = ctx.enter_context(nc.sbuf_tensor("out_big", [P, 2, 4, Dv], f32))
        acc = [
            ctx.enter_context(nc.psum_tensor(f"acc{i}", [P, 4, 2, Dv], f32))
            for i in range(2)
        ]
        ksem = [ctx.enter_context(nc.semaphore(name=f"ks{i}")) for i in range(len(K_PIECES))]
        vsem = [ctx.enter_context(nc.semaphore(name=f"vs{i}")) for i in range(len(V_PIECES))]
        dve_sem = ctx.enter_context(nc.semaphore(name="dve_sem"))
        pe_sem = ctx.enter_context(nc.semaphore(name="pe_sem"))
        out_sem = ctx.enter_context(nc.semaphore(name="out_sem"))
        done_sem = ctx.enter_context(nc.semaphore(name="done_sem"))
        block = ctx.enter_context(nc.Block())

        @block.sync
        def _(sync):
            for vi, (a, b) in enumerate(V_PIECES):
                sync.dma_start(vt[:, a:b, :], vals_d[:, a:b, :]).then_inc(
                    vsem[vi], 16
                )
            sync.wait_ge(dve_sem, 5)
            sync.dma_start(out_d[:, 256:512], out_big[:, 1, :, :]).then_inc(
                out_sem, 16
            )
            sync.wait_ge(out_sem, 32)
            sync.nop().then_inc(done_sem, 1)

        @block.gpsimd
        def _(gpsimd):
            # Semaphores persist across NEFF executions; clear them all at
            # the end (after every engine is provably done) so the kernel
            # is safe to run repeatedly.  (Clearing the input semaphores
            # earlier, hidden under the output receipt, measured SLOWER --
            # 31.9us vs 30.1us -- so the simple done-gated chain stays.)
            gpsimd.wait_ge(done_sem, 1)
            for s in [*ksem, *vsem, dve_sem, pe_sem, out_sem, done_sem]:
                gpsimd.sem_clear(s)

        @block.scalar
        def _(scalar):
            # Keys stream on ACT's HWDGE ring, in parallel with values on SP.
            for ki, (a, b) in enumerate(K_PIECES):
                scalar.dma_start(kt[:, a:b, :], keys_d[:, a:b, :]).then_inc(
                    ksem[ki], 16
                )
            # Then the bank-0 half of the output, while DVE still evacuates
            # bank 1.  ACT does no datapath work (an ACT copy feeding an ACT
            # dma_start raced -- the sequencer-level DMA trigger does not
            # order against the in-flight datapath op).
            scalar.wait_ge(dve_sem, 3)
            scalar.dma_start(out_d[:, 0:256], out_big[:, 0, :, :]).then_inc(
                out_sem, 16
            )

        @block.vector
        def _(vector):
            vector.memset(acc[0][:], 0.0)
            vector.memset(acc[1][:], 0.0).then_inc(dve_sem, 1)
            # bank 0 strips while PE finishes bank 1 (different banks)
            vector.wait_ge(pe_sem, 16)
            vector.tensor_copy(out_big[0:64, 0, :, :], acc[0][0:64, :, 0, :]).then_inc(
                dve_sem, 1
            )
            vector.tensor_copy(
                out_big[64:128, 0, :, :], acc[0][64:128, :, 1, :]
            ).then_inc(dve_sem, 1)
            vector.wait_ge(pe_sem, 17)
            vector.tensor_copy(out_big[0:64, 1, :, :], acc[1][0:64, :, 0, :]).then_inc(
                dve_sem, 1
            )
            vector.tensor_copy(
                out_big[64:128, 1, :, :], acc[1][64:128, :, 1, :]
            ).then_inc(dve_sem, 1)

        @block.tensor
        def _(tensor):
            last_k = last_v = -1
            for c in range(CHUNKS):
                if k_of[c] != last_k:
                    last_k = k_of[c]
                    tensor.wait_ge(ksem[last_k], 16)
                if v_of[c] != last_v:
                    last_v = v_of[c]
                    tensor.wait_ge(vsem[last_v], 16)
                if c == 0:
                    tensor.wait_ge(dve_sem, 1)  # memsets done
                last = c == CHUNKS - 1
                for q in range(NPAIR):
                    g, qq = divmod(q, 4)
                    mm = tensor.matmul(
                        acc[g][:, qq, :, :],
                        kt[:, c, q * 128 : (q + 1) * 128],
                        vt[:, c, q * 128 : (q + 1) * 128],
                        start=False,
                        stop=last and (q == 3 or q == NPAIR - 1),
                        skip_group_check=True,
                    )
                    if (last and q == 3) or q == NPAIR - 1:
                        mm.then_inc(pe_sem, 1)

    return nc


def _get_nc():
    global _nc_cache
    if _nc_cache is None:
        _nc_cache = _build_nc()
    return _nc_cache


def _make_in_maps(keys, values, write_strengths):
    # Fold sqrt(rho) into both operands in fp32, one fp8 cast for the full
    # arrays, then per-core (chunk, p, f) -> (p, chunk, f) transposes so
    # each DMA partition line is contiguous in DRAM.
    sq = np.sqrt(np.asarray(write_strengths, dtype=np.float32)).reshape(B * S, 1)
    kq = (keys.reshape(B * S, FD) * sq).astype(F8)
    vq = (values.reshape(B * S, FD) * sq).astype(F8)
    in_maps = []
    for c in range(N_CORES):
        sl = slice(c * NS, (c + 1) * NS)
        in_maps.append(
            {
                "keys": np.ascontiguousarray(
                    kq[sl].reshape(CHUNKS, P, FD).transpose(1, 0, 2)
                ),
                "values": np.ascontiguousarray(
                    vq[sl].reshape(CHUNKS, P, FD).transpose(1, 0, 2)
                ),
            }
        )
    return in_maps


def _run(in_maps, **kwargs):
    from concourse.bass_utils import run_bass_kernel_spmd

    nc = _get_nc()
    return run_bass_kernel_spmd(nc, in_maps, core_ids=list(range(N_CORES)), **kwargs)


def _assemble(memory, results):
    parts = np.stack([r["delta"] for r in results], axis=0)  # (8, 128, 512)
    arr = parts.sum(axis=0, dtype=np.float64)  # (128, 512)
    blk = arr.reshape(128, 2, 4, Dv)
    delta_hkv = np.empty((H, Dk, Dv))
    for g in range(2):
        for q in range(4):
            delta_hkv[2 * (4 * g + q)] = blk[0:64, g, q, :]
            delta_hkv[2 * (4 * g + q) + 1] = blk[64:128, g, q, :]
    out = DECAY * np.asarray(memory, dtype=np.float64) + delta_hkv
    return out.astype(np.float32)


def kernel(memory, keys, values, write_strengths):
    memory = np.asarray(memory, dtype=np.float32)
    keys = np.asarray(keys, dtype=np.float32)
    values = np.asarray(values, dtype=np.float32)
    write_strengths = np.asarray(write_strengths, dtype=np.float32)

    in_maps = _make_in_maps(keys, values, write_strengths)
    res = _run(in_maps)
    return _assemble(memory, res.results)


if __name__ == "__main__":
    rng = np.random.default_rng(0)
    mem = rng.standard_normal((H, Dk, Dv), dtype=np.float32)
    k = rng.standard_normal((B, S, H, Dk), dtype=np.float32)
    v = rng.standard_normal((B, S, H, Dv), dtype=np.float32)
    w = rng.random((B, S), dtype=np.float32)
    out = kernel(mem, k, v, w)
    ref = DECAY * mem + np.einsum(
        "bs,bshk,bshv->hkv", w.astype(np.float64), k.astype(np.float64), v.astype(np.float64)
    )
    err = np.abs(out - ref).max() / np.abs(ref).max()
    print("self-check rel err:", err)


# revision 47
# speedup vs baseline: 1.0853x; 1.0768x over previous
"""Trainium2 Bass kernel for the L1Writer scatter-memory problem.

Computes   out = 0.95 * memory + einsum('bs,bshk,bshv->hkv', rho, keys, values)

Strategy: data-parallel over the flattened (B*S)=16384 token axis, 2048 rows
per core.  Each core computes its partial delta
    delta_h = (sqrt(rho) K_h)^T (sqrt(rho) V_h)     (per head h)
as a chain of 128-row PE matmuls accumulating in PSUM.  The 8 partial
(H,Dk,Dv) deltas are summed on the host (tiny: 256 KB each) and added to
decay*memory there.

sqrt(rho) is folded into BOTH keys and values on the host and the results
are cast to fp8 e4m3 (PSUM accumulation stays fp32; fp8 products are exact
in fp32, so the device result matches a host simulation of the quantized
inputs).  Measured end-to-end rel err 7.2e-3 on the fixed reference inputs,
inside the 2e-2 gate with 2.8x margin.  fp8 quarters HBM traffic vs fp32 --
the kernel is memory-bound -- and enables 128-column weight loads (two
adjacent heads per LDWEIGHTS) to halve the PE's weight-load bandwidth floor.

Per-core kernel layout:
  - Keys and values are interleaved on the host into one (128, 16, 2, 1024)
    fp8 tensor: slot s holds chunk 2s (s<8) or 2(s-8)+1 (s>=8), with
    [p, s, 0, :] = keys and [p, s, 1, :] = values of token c*128+p.  Slots
    0-7 (even chunks) stream on the SP HWDGE ring, slots 8-15 (odd chunks)
    on the ACT ring: the two descriptor generators run in parallel (one
    ring can't stay fed -- generation costs ~0.6-1.1us per trigger), every
    piece is >=256KB so drain never outpaces generation, and since both
    rings drain at the same rate the chunks arrive globally in order.
    One semaphore per DMA piece covers that chunk range's keys AND values
    (a shared counting semaphore across DMAs is unsound -- 16 SDMA
    engines inc independently and drain at different rates).
  - Per chunk c and head pair q: one LDWEIGHTS of K[:, heads 2q,2q+1]
    (128 columns) and one 128-wide matmul against V[:, heads 2q,2q+1]
    accumulating into acc[q//4][:, q%4, :, :] ([128, 4, 2, 64] per PSUM
    bank).  Rows 0-63 of block half 0 hold head 2q's delta; rows 64-127
    of half 1 hold head 2q+1's; the other two quadrants are don't-care
    cross products.  All matmuls use start=False onto DVE-memset zeros
    (first write per element overwrites or accumulates onto zero --
    correct for any stale has_written bits).
  - The last chunk runs bank-0 pairs first: DVE evacuates bank 0's two
    valid quadrant strips (partition-aligned copies) while PE finishes
    bank 1, ACT DMAs the bank-0 half of the output (its own HWDGE ring)
    while DVE copies bank 1, then sync DMAs the bank-1 half.  DRAM
    delta is (128, 512) fp32 = [k | 64+k][g][q][v]; host reassembles
    (H, Dk, Dv) with h = 2*(4g+q) (+1 for rows 64-127).
"""

import numpy as np
import ml_dtypes

F8 = ml_dtypes.float8_e4m3   # matches mybir.dt.float8e4 on this platform

DECAY = 0.95
B, S, H, Dk, Dv = 4, 4096, 16, 64, 64
N_CORES = 8
NS = (B * S) // N_CORES          # 2048 rows per core
P = 128                          # partitions
CHUNKS = NS // P                 # 16 contraction chunks of 128 rows
FD = H * Dk                      # 1024 features per row
NPAIR = H // 2                   # 8 head pairs per chunk

# slot s carries chunk CHUNK_OF[s]; chunk c lives at slot SLOT_OF[c]
CHUNK_OF = [2 * s for s in range(8)] + [2 * s + 1 for s in range(8)]
SLOT_OF = [CHUNK_OF.index(c) for c in range(CHUNKS)]

# (start_slot, end_slot) DMA pieces per ring; 1-slot pieces at the tail so
# the post-DMA burst is small.
SP_PIECES = [(0, 3), (3, 6), (6, 7), (7, 8)]       # even chunks 0..14
ACT_PIECES = [(8, 11), (11, 14), (14, 15), (15, 16)]  # odd chunks 1..15

_nc_cache = None


def _build_nc():
    from contextlib import ExitStack

    import concourse.bass as bass
    import concourse.mybir as mybir

    f32 = mybir.dt.float32
    f8 = mybir.dt.float8e4
    nc = bass.Bass()

    kv_d = nc.dram_tensor("kv", (P, CHUNKS, 2, FD), f8, kind="ExternalInput")
    out_d = nc.dram_tensor("delta", (P, 512), f32, kind="ExternalOutput")

    pieces = SP_PIECES + ACT_PIECES
    # slot -> piece index
    p_of = [next(i for i, (a, b) in enumerate(pieces) if a <= s < b) for s in range(CHUNKS)]

    # Raw bass (no Tile); all waits are standalone sequencer wait_ge ops.
    #
    # Semaphore budget: kvsem[8] (one per DMA piece, exact completion) +
    # dve_sem (1 memset + 4 evac copies) + pe_sem (+1 per chunk 0..14, +1
    # after last chunk's bank-0 pairs, +1 after bank-1 pairs) + out_sem
    # (+16 per output half; sync waits 32) + done_sem.
    with ExitStack() as ctx:
        kvt = ctx.enter_context(nc.sbuf_tensor("kvt", [P, CHUNKS, 2, FD], f8))
        # out_big[p, g, q, v]: bank-g pair-q quadrant strips
        out_big = ctx.enter_context(nc.sbuf_tensor("out_big", [P, 2, 4, Dv], f32))
        acc = [
            ctx.enter_context(nc.psum_tensor(f"acc{i}", [P, 4, 2, Dv], f32))
            for i in range(2)
        ]
        kvsem = [ctx.enter_context(nc.semaphore(name=f"kv{i}")) for i in range(len(pieces))]
        dve_sem = ctx.enter_context(nc.semaphore(name="dve_sem"))
        pe_sem = ctx.enter_context(nc.semaphore(name="pe_sem"))
        out_sem = ctx.enter_context(nc.semaphore(name="out_sem"))
        done_sem = ctx.enter_context(nc.semaphore(name="done_sem"))
        block = ctx.enter_context(nc.Block())

        @block.sync
        def _(sync):
            for pi, (a, b) in enumerate(SP_PIECES):
                sync.dma_start(kvt[:, a:b, :, :], kv_d[:, a:b, :, :]).then_inc(
                    kvsem[pi], 16
                )
            sync.wait_ge(dve_sem, 5)
            sync.dma_start(out_d[:, 256:512], out_big[:, 1, :, :]).then_inc(
                out_sem, 16
            )
            sync.wait_ge(out_sem, 32)
            sync.nop().then_inc(done_sem, 1)

        @block.gpsimd
        def _(gpsimd):
            # Semaphores persist across NEFF executions; clear them all at
            # the end (after every engine is provably done) so the kernel
            # is safe to run repeatedly.
            gpsimd.wait_ge(done_sem, 1)
            for s in [*kvsem, dve_sem, pe_sem, out_sem, done_sem]:
                gpsimd.sem_clear(s)

        @block.scalar
        def _(scalar):
            # Odd chunks stream on ACT's HWDGE ring, in parallel with SP.
            for pi, (a, b) in enumerate(ACT_PIECES):
                scalar.dma_start(kvt[:, a:b, :, :], kv_d[:, a:b, :, :]).then_inc(
                    kvsem[len(SP_PIECES) + pi], 16
                )
            # Then the bank-0 half of the output, while DVE still evacuates
            # bank 1.  ACT does no datapath work (an ACT copy feeding an ACT
            # dma_start raced -- the sequencer-level DMA trigger does not
            # order against the in-flight datapath op).
            scalar.wait_ge(dve_sem, 3)
            scalar.dma_start(out_d[:, 0:256], out_big[:, 0, :, :]).then_inc(
                out_sem, 16
            )

        @block.vector
        def _(vector):
            vector.memset(acc[0][:], 0.0)
            vector.memset(acc[1][:], 0.0).then_inc(dve_sem, 1)
            # bank 0 strips while PE finishes bank 1 (different banks)
            vector.wait_ge(pe_sem, 16)
            vector.tensor_copy(out_big[0:64, 0, :, :], acc[0][0:64, :, 0, :]).then_inc(
                dve_sem, 1
            )
            vector.tensor_copy(
                out_big[64:128, 0, :, :], acc[0][64:128, :, 1, :]
            ).then_inc(dve_sem, 1)
            vector.wait_ge(pe_sem, 17)
            vector.tensor_copy(out_big[0:64, 1, :, :], acc[1][0:64, :, 0, :]).then_inc(
                dve_sem, 1
            )
            vector.tensor_copy(
                out_big[64:128, 1, :, :], acc[1][64:128, :, 1, :]
            ).then_inc(dve_sem, 1)

        @block.tensor
        def _(tensor):
            waited = set()
            for c in range(CHUNKS):
                s = SLOT_OF[c]
                pi = p_of[s]
                if pi not in waited:
                    waited.add(pi)
                    tensor.wait_ge(kvsem[pi], 16)
                if c == 0:
                    tensor.wait_ge(dve_sem, 1)  # memsets done
                last = c == CHUNKS - 1
                for q in range(NPAIR):
                    g, qq = divmod(q, 4)
                    mm = tensor.matmul(
                        acc[g][:, qq, :, :],
                        kvt[:, s, 0, q * 128 : (q + 1) * 128],
                        kvt[:, s, 1, q * 128 : (q + 1) * 128],
                        start=False,
                        stop=last and (q == 3 or q == NPAIR - 1),
                        skip_group_check=True,
                    )
                    if (last and q == 3) or q == NPAIR - 1:
                        mm.then_inc(pe_sem, 1)

    return nc


def _get_nc():
    global _nc_cache
    if _nc_cache is None:
        _nc_cache = _build_nc()
    return _nc_cache


def _make_in_maps(keys, values, write_strengths):
    # Fold sqrt(rho) into both operands in fp32, one fp8 cast for the full
    # arrays, then per-core: interleave chunk-permuted keys/values into the
    # (p, slot, 2, f) layout so each DMA partition line is contiguous.
    sq = np.sqrt(np.asarray(write_strengths, dtype=np.float32)).reshape(B * S, 1)
    kq = (keys.reshape(B * S, FD) * sq).astype(F8)
    vq = (values.reshape(B * S, FD) * sq).astype(F8)
    in_maps = []
    for c in range(N_CORES):
        sl = slice(c * NS, (c + 1) * NS)
        kc = kq[sl].reshape(CHUNKS, P, FD)
        vc = vq[sl].reshape(CHUNKS, P, FD)
        kv = np.stack([kc, vc], axis=1)[CHUNK_OF]      # (slot, 2, P, FD)
        in_maps.append(
            {"kv": np.ascontiguousarray(kv.transpose(2, 0, 1, 3))}
        )
    return in_maps


def _run(in_maps, **kwargs):
    from concourse.bass_utils import run_bass_kernel_spmd

    nc = _get_nc()
    return run_bass_kernel_spmd(nc, in_maps, core_ids=list(range(N_CORES)), **kwargs)


def _assemble(memory, results):
    parts = np.stack([r["delta"] for r in results], axis=0)  # (8, 128, 512)
    arr = parts.sum(axis=0, dtype=np.float64)  # (128, 512)
    blk = arr.reshape(128, 2, 4, Dv)
    delta_hkv = np.empty((H, Dk, Dv))
    for g in range(2):
        for q in range(4):
            delta_hkv[2 * (4 * g + q)] = blk[0:64, g, q, :]
            delta_hkv[2 * (4 * g + q) + 1] = blk[64:128, g, q, :]
    out = DECAY * np.asarray(memory, dtype=np.float64) + delta_hkv
    return out.astype(np.float32)


def kernel(memory, keys, values, write_strengths):
    memory = np.asarray(memory, dtype=np.float32)
    keys = np.asarray(keys, dtype=np.float32)
    values = np.asarray(values, dtype=np.float32)
    write_strengths = np.asarray(write_strengths, dtype=np.float32)

    in_maps = _make_in_maps(keys, values, write_strengths)
    res = _run(in_maps)
    return _assemble(memory, res.results)


if __name__ == "__main__":
    rng = np.random.default_rng(0)
    mem = rng.standard_normal((H, Dk, Dv), dtype=np.float32)
    k = rng.standard_normal((B, S, H, Dk), dtype=np.float32)
    v = rng.standard_normal((B, S, H, Dv), dtype=np.float32)
    w = rng.random((B, S), dtype=np.float32)
    out = kernel(mem, k, v, w)
    ref = DECAY * mem + np.einsum(
        "bs,bshk,bshv->hkv", w.astype(np.float64), k.astype(np.float64), v.astype(np.float64)
    )
    err = np.abs(out - ref).max() / np.abs(ref).max()
    print("self-check rel err:", err)


# revision 48
# speedup vs baseline: 1.1497x; 1.0594x over previous
"""Trainium2 Bass kernel for the L1Writer scatter-memory problem.

Computes   out = 0.95 * memory + einsum('bs,bshk,bshv->hkv', rho, keys, values)

Strategy: data-parallel over the flattened (B*S)=16384 token axis, 2048 rows
per core.  Each core computes its partial delta
    delta_h = (sqrt(rho) K_h)^T (sqrt(rho) V_h)     (per head h)
as a chain of 128-row PE matmuls accumulating in PSUM.  The 8 partial
(H,Dk,Dv) deltas are summed on the host (tiny: 256 KB each) and added to
decay*memory there.

sqrt(rho) is folded into BOTH keys and values on the host and the results
are cast to fp8 e4m3 (PSUM accumulation stays fp32; fp8 products are exact
in fp32, so the device result matches a host simulation of the quantized
inputs).  Measured end-to-end rel err 7.2e-3 on the fixed reference inputs,
inside the 2e-2 gate with 2.8x margin.  fp8 quarters HBM traffic vs fp32 --
the kernel is memory-bound -- and enables 128-column weight loads (two
adjacent heads per LDWEIGHTS) to halve the PE's weight-load bandwidth floor.

Per-core kernel layout:
  - Keys and values are interleaved on the host into one (128, 16, 2, 1024)
    fp8 tensor: slot s holds chunk 2s (s<8) or 2(s-8)+1 (s>=8), with
    [p, s, 0, :] = keys and [p, s, 1, :] = values of token c*128+p.  Slots
    0-7 (even chunks) stream on the SP HWDGE ring, slots 8-15 (odd chunks)
    on the ACT ring: the two descriptor generators run in parallel (one
    ring can't stay fed -- generation costs ~0.6-1.1us per trigger), every
    piece is >=256KB so drain never outpaces generation, and since both
    rings drain at the same rate the chunks arrive globally in order.
    One semaphore per DMA piece covers that chunk range's keys AND values
    (a shared counting semaphore across DMAs is unsound -- 16 SDMA
    engines inc independently and drain at different rates).
  - Per chunk c and head pair q: one LDWEIGHTS of K[:, heads 2q,2q+1]
    (128 columns) and one 128-wide matmul against V[:, heads 2q,2q+1]
    accumulating into acc[q//4][:, q%4, :, :] ([128, 4, 2, 64] per PSUM
    bank).  Rows 0-63 of block half 0 hold head 2q's delta; rows 64-127
    of half 1 hold head 2q+1's; the other two quadrants are don't-care
    cross products.  All matmuls use start=False onto DVE-memset zeros
    (first write per element overwrites or accumulates onto zero --
    correct for any stale has_written bits).
  - The last chunk runs bank-0 pairs first: DVE evacuates bank 0's two
    valid quadrant strips (partition-aligned copies) while PE finishes
    bank 1, ACT DMAs the bank-0 half of the output (its own HWDGE ring)
    while DVE copies bank 1, then sync DMAs the bank-1 half.  DRAM
    delta is (128, 512) fp32 = [k | 64+k][g][q][v]; host reassembles
    (H, Dk, Dv) with h = 2*(4g+q) (+1 for rows 64-127).
"""

import numpy as np
import ml_dtypes

F8 = ml_dtypes.float8_e4m3   # matches mybir.dt.float8e4 on this platform

DECAY = 0.95
B, S, H, Dk, Dv = 4, 4096, 16, 64, 64
N_CORES = 8
NS = (B * S) // N_CORES          # 2048 rows per core
P = 128                          # partitions
CHUNKS = NS // P                 # 16 contraction chunks of 128 rows
FD = H * Dk                      # 1024 features per row
NPAIR = H // 2                   # 8 head pairs per chunk

# slot s carries chunk CHUNK_OF[s]; chunk c lives at slot SLOT_OF[c]
CHUNK_OF = [2 * s for s in range(8)] + [2 * s + 1 for s in range(8)]
SLOT_OF = [CHUNK_OF.index(c) for c in range(CHUNKS)]

# (start_slot, end_slot) DMA pieces per ring; 1-slot pieces at the tail so
# the post-DMA burst is small.
SP_PIECES = [(0, 3), (3, 6), (6, 7), (7, 8)]       # even chunks 0..14
ACT_PIECES = [(8, 11), (11, 14), (14, 15), (15, 16)]  # odd chunks 1..15

_nc_cache = None


def _build_nc():
    from contextlib import ExitStack

    import concourse.bass as bass
    import concourse.mybir as mybir

    f32 = mybir.dt.float32
    f8 = mybir.dt.float8e4
    nc = bass.Bass()

    kv_d = nc.dram_tensor("kv", (P, CHUNKS, 2, FD), f8, kind="ExternalInput")
    out_d = nc.dram_tensor("delta", (P, 512), f32, kind="ExternalOutput")

    pieces = SP_PIECES + ACT_PIECES
    # slot -> piece index
    p_of = [next(i for i, (a, b) in enumerate(pieces) if a <= s < b) for s in range(CHUNKS)]

    # Raw bass (no Tile); all waits are standalone sequencer wait_ge ops.
    #
    # Semaphore budget: kvsem[8] (one per DMA piece, exact completion) +
    # dve_sem (1 memset + 4 evac copies) + pe_sem (+1 per chunk 0..14, +1
    # after last chunk's bank-0 pairs, +1 after bank-1 pairs) + out_sem
    # (+16 per output half; sync waits 32) + done_sem.
    with ExitStack() as ctx:
        kvt = ctx.enter_context(nc.sbuf_tensor("kvt", [P, CHUNKS, 2, FD], f8))
        # out_big[p, g, q, v]: bank-g pair-q quadrant strips
        out_big = ctx.enter_context(nc.sbuf_tensor("out_big", [P, 2, 4, Dv], f32))
        acc = [
            ctx.enter_context(nc.psum_tensor(f"acc{i}", [P, 4, 2, Dv], f32))
            for i in range(2)
        ]
        kvsem = [ctx.enter_context(nc.semaphore(name=f"kv{i}")) for i in range(len(pieces))]
        dve_sem = ctx.enter_context(nc.semaphore(name="dve_sem"))
        pe_sem = ctx.enter_context(nc.semaphore(name="pe_sem"))
        out_sem = ctx.enter_context(nc.semaphore(name="out_sem"))
        done_sem = ctx.enter_context(nc.semaphore(name="done_sem"))
        block = ctx.enter_context(nc.Block())

        @block.sync
        def _(sync):
            for pi, (a, b) in enumerate(SP_PIECES):
                sync.dma_start(kvt[:, a:b, :, :], kv_d[:, a:b, :, :]).then_inc(
                    kvsem[pi], 16
                )
            sync.wait_ge(dve_sem, 5)
            sync.dma_start(out_d[:, 256:512], out_big[:, 1, :, :]).then_inc(
                out_sem, 16
            )
            sync.wait_ge(out_sem, 32)
            sync.nop().then_inc(done_sem, 1)

        @block.gpsimd
        def _(gpsimd):
            # Semaphores persist across NEFF executions; clear them all at
            # the end (after every engine is provably done) so the kernel
            # is safe to run repeatedly.
            gpsimd.wait_ge(done_sem, 1)
            for s in [*kvsem, dve_sem, pe_sem, out_sem, done_sem]:
                gpsimd.sem_clear(s)

        @block.scalar
        def _(scalar):
            # Odd chunks stream on ACT's HWDGE ring, in parallel with SP.
            for pi, (a, b) in enumerate(ACT_PIECES):
                scalar.dma_start(kvt[:, a:b, :, :], kv_d[:, a:b, :, :]).then_inc(
                    kvsem[len(SP_PIECES) + pi], 16
                )
            # Then the bank-0 half of the output, while DVE still evacuates
            # bank 1.  ACT does no datapath work (an ACT copy feeding an ACT
            # dma_start raced -- the sequencer-level DMA trigger does not
            # order against the in-flight datapath op).
            scalar.wait_ge(dve_sem, 3)
            scalar.dma_start(out_d[:, 0:256], out_big[:, 0, :, :]).then_inc(
                out_sem, 16
            )

        @block.vector
        def _(vector):
            vector.memset(acc[0][:], 0.0)
            vector.memset(acc[1][:], 0.0).then_inc(dve_sem, 1)
            # bank 0 strips while PE finishes bank 1 (different banks)
            vector.wait_ge(pe_sem, 16)
            vector.tensor_copy(out_big[0:64, 0, :, :], acc[0][0:64, :, 0, :]).then_inc(
                dve_sem, 1
            )
            vector.tensor_copy(
                out_big[64:128, 0, :, :], acc[0][64:128, :, 1, :]
            ).then_inc(dve_sem, 1)
            vector.wait_ge(pe_sem, 17)
            vector.tensor_copy(out_big[0:64, 1, :, :], acc[1][0:64, :, 0, :]).then_inc(
                dve_sem, 1
            )
            vector.tensor_copy(
                out_big[64:128, 1, :, :], acc[1][64:128, :, 1, :]
            ).then_inc(dve_sem, 1)

        @block.tensor
        def _(tensor):
            # PSUM accumulation is commutative, so chunks are processed
            # piece-by-piece in expected piece-ARRIVAL order (rings
            # alternating), draining each whole piece before the next wait.
            # Consuming in numeric chunk order ping-pongs between the two
            # rings every chunk and stalls on whichever ring is behind --
            # the rings were measured draining with up to 3us of skew.
            piece_order = [0, 4, 1, 5, 2, 6, 3, 7]
            processed = 0
            for pi in piece_order:
                a, b = pieces[pi]
                tensor.wait_ge(kvsem[pi], 16)
                if processed == 0:
                    tensor.wait_ge(dve_sem, 1)  # memsets done
                for s in range(a, b):
                    processed += 1
                    last = processed == CHUNKS
                    for q in range(NPAIR):
                        g, qq = divmod(q, 4)
                        mm = tensor.matmul(
                            acc[g][:, qq, :, :],
                            kvt[:, s, 0, q * 128 : (q + 1) * 128],
                            kvt[:, s, 1, q * 128 : (q + 1) * 128],
                            start=False,
                            stop=last and (q == 3 or q == NPAIR - 1),
                            skip_group_check=True,
                        )
                        if (last and q == 3) or q == NPAIR - 1:
                            mm.then_inc(pe_sem, 1)

    return nc


def _get_nc():
    global _nc_cache
    if _nc_cache is None:
        _nc_cache = _build_nc()
    return _nc_cache


def _make_in_maps(keys, values, write_strengths):
    # Fold sqrt(rho) into both operands in fp32, one fp8 cast for the full
    # arrays, then per-core: interleave chunk-permuted keys/values into the
    # (p, slot, 2, f) layout so each DMA partition line is contiguous.
    sq = np.sqrt(np.asarray(write_strengths, dtype=np.float32)).reshape(B * S, 1)
    kq = (keys.reshape(B * S, FD) * sq).astype(F8)
    vq = (values.reshape(B * S, FD) * sq).astype(F8)
    in_maps = []
    for c in range(N_CORES):
        sl = slice(c * NS, (c + 1) * NS)
        kc = kq[sl].reshape(CHUNKS, P, FD)
        vc = vq[sl].reshape(CHUNKS, P, FD)
        kv = np.stack([kc, vc], axis=1)[CHUNK_OF]      # (slot, 2, P, FD)
        in_maps.append(
            {"kv": np.ascontiguousarray(kv.transpose(2, 0, 1, 3))}
        )
    return in_maps


def _run(in_maps, **kwargs):
    from concourse.bass_utils import run_bass_kernel_spmd

    nc = _get_nc()
    return run_bass_kernel_spmd(nc, in_maps, core_ids=list(range(N_CORES)), **kwargs)


def _assemble(memory, results):
    parts = np.stack([r["delta"] for r in results], axis=0)  # (8, 128, 512)
    arr = parts.sum(axis=0, dtype=np.float64)  # (128, 512)
    blk = arr.reshape(128, 2, 4, Dv)
    delta_hkv = np.empty((H, Dk, Dv))
    for g in range(2):
        for q in range(4):
            delta_hkv[2 * (4 * g + q)] = blk[0:64, g, q, :]
            delta_hkv[2 * (4 * g + q) + 1] = blk[64:128, g, q, :]
    out = DECAY * np.asarray(memory, dtype=np.float64) + delta_hkv
    return out.astype(np.float32)


def kernel(memory, keys, values, write_strengths):
    memory = np.asarray(memory, dtype=np.float32)
    keys = np.asarray(keys, dtype=np.float32)
    values = np.asarray(values, dtype=np.float32)
    write_strengths = np.asarray(write_strengths, dtype=np.float32)

    in_maps = _make_in_maps(keys, values, write_strengths)
    res = _run(in_maps)
    return _assemble(memory, res.results)


if __name__ == "__main__":
    rng = np.random.default_rng(0)
    mem = rng.standard_normal((H, Dk, Dv), dtype=np.float32)
    k = rng.standard_normal((B, S, H, Dk), dtype=np.float32)
    v = rng.standard_normal((B, S, H, Dv), dtype=np.float32)
    w = rng.random((B, S), dtype=np.float32)
    out = kernel(mem, k, v, w)
    ref = DECAY * mem + np.einsum(
        "bs,bshk,bshv->hkv", w.astype(np.float64), k.astype(np.float64), v.astype(np.float64)
    )
    err = np.abs(out - ref).max() / np.abs(ref).max()
    print("self-check rel err:", err)


# revision 51
# speedup vs baseline: 1.1521x; 1.0020x over previous
"""Trainium2 Bass kernel for the L1Writer scatter-memory problem.

Computes   out = 0.95 * memory + einsum('bs,bshk,bshv->hkv', rho, keys, values)

Strategy: data-parallel over the flattened (B*S)=16384 token axis, 2048 rows
per core.  Each core computes its partial delta
    delta_h = (sqrt(rho) K_h)^T (sqrt(rho) V_h)     (per head h)
as a chain of 128-row PE matmuls accumulating in PSUM.  The 8 partial
(H,Dk,Dv) deltas are summed on the host (tiny: 256 KB each) and added to
decay*memory there.

sqrt(rho) is folded into BOTH keys and values on the host and the results
are cast to fp8 e4m3 (PSUM accumulation stays fp32; fp8 products are exact
in fp32, so the device result matches a host simulation of the quantized
inputs).  Measured end-to-end rel err 7.2e-3 on the fixed reference inputs,
inside the 2e-2 gate with 2.8x margin.  fp8 quarters HBM traffic vs fp32 --
the kernel is memory-bound -- and enables 128-column weight loads (two
adjacent heads per LDWEIGHTS) to halve the PE's weight-load bandwidth floor.

Per-core kernel layout:
  - Keys and values are interleaved on the host into one (128, 16, 2, 1024)
    fp8 tensor: slot s holds chunk 2s (s<8) or 2(s-8)+1 (s>=8), with
    [p, s, 0, :] = keys and [p, s, 1, :] = values of token c*128+p.  Slots
    0-7 (even chunks) stream on the SP HWDGE ring, slots 8-15 (odd chunks)
    on the ACT ring: the two descriptor generators run in parallel (one
    ring can't stay fed -- generation costs ~0.6-1.1us per trigger), every
    piece is >=256KB so drain never outpaces generation, and since both
    rings drain at the same rate the chunks arrive globally in order.
    One semaphore per DMA piece covers that chunk range's keys AND values
    (a shared counting semaphore across DMAs is unsound -- 16 SDMA
    engines inc independently and drain at different rates).
  - Per chunk c and head pair q: one LDWEIGHTS of K[:, heads 2q,2q+1]
    (128 columns) and one 128-wide matmul against V[:, heads 2q,2q+1]
    accumulating into acc[q//4][:, q%4, :, :] ([128, 4, 2, 64] per PSUM
    bank).  Rows 0-63 of block half 0 hold head 2q's delta; rows 64-127
    of half 1 hold head 2q+1's; the other two quadrants are don't-care
    cross products.  All matmuls use start=False onto DVE-memset zeros
    (first write per element overwrites or accumulates onto zero --
    correct for any stale has_written bits).
  - The last chunk runs bank-0 pairs first: DVE evacuates bank 0's two
    valid quadrant strips (partition-aligned copies) while PE finishes
    bank 1, ACT DMAs the bank-0 half of the output (its own HWDGE ring)
    while DVE copies bank 1, then sync DMAs the bank-1 half.  DRAM
    delta is (128, 512) fp32 = [k | 64+k][g][q][v]; host reassembles
    (H, Dk, Dv) with h = 2*(4g+q) (+1 for rows 64-127).
"""

import numpy as np
import ml_dtypes

F8 = ml_dtypes.float8_e4m3   # matches mybir.dt.float8e4 on this platform

DECAY = 0.95
B, S, H, Dk, Dv = 4, 4096, 16, 64, 64
N_CORES = 8
NS = (B * S) // N_CORES          # 2048 rows per core
P = 128                          # partitions
CHUNKS = NS // P                 # 16 contraction chunks of 128 rows
FD = H * Dk                      # 1024 features per row
NPAIR = H // 2                   # 8 head pairs per chunk

# slot s carries chunk CHUNK_OF[s]; chunk c lives at slot SLOT_OF[c]
CHUNK_OF = [2 * s for s in range(8)] + [2 * s + 1 for s in range(8)]
SLOT_OF = [CHUNK_OF.index(c) for c in range(CHUNKS)]

# (start_slot, end_slot) DMA pieces per ring; 1-slot pieces at the tail so
# the post-DMA burst is small.
SP_PIECES = [(0, 3), (3, 6), (6, 7), (7, 8)]       # even chunks 0..14
ACT_PIECES = [(8, 11), (11, 14), (14, 15), (15, 16)]  # odd chunks 1..15

_nc_cache = None


def _build_nc():
    from contextlib import ExitStack

    import concourse.bass as bass
    import concourse.mybir as mybir

    f32 = mybir.dt.float32
    f8 = mybir.dt.float8e4
    nc = bass.Bass()

    kv_d = nc.dram_tensor("kv", (P, CHUNKS, 2, FD), f8, kind="ExternalInput")
    out_d = nc.dram_tensor("delta", (P, 512), f32, kind="ExternalOutput")

    pieces = SP_PIECES + ACT_PIECES
    # slot -> piece index
    p_of = [next(i for i, (a, b) in enumerate(pieces) if a <= s < b) for s in range(CHUNKS)]

    # Raw bass (no Tile); all waits are standalone sequencer wait_ge ops.
    #
    # Semaphore budget: kvsem[8] (one per DMA piece, exact completion) +
    # dve_sem (1 memset + 4 evac copies) + pe_sem (+1 per chunk 0..14, +1
    # after last chunk's bank-0 pairs, +1 after bank-1 pairs) + out_sem
    # (+16 per output piece x3; sync waits 48) + done_sem.
    with ExitStack() as ctx:
        kvt = ctx.enter_context(nc.sbuf_tensor("kvt", [P, CHUNKS, 2, FD], f8))
        # out_big[p, g, q, v]: bank-g pair-q quadrant strips
        out_big = ctx.enter_context(nc.sbuf_tensor("out_big", [P, 2, 4, Dv], f32))
        acc = [
            ctx.enter_context(nc.psum_tensor(f"acc{i}", [P, 4, 2, Dv], f32))
            for i in range(2)
        ]
        kvsem = [ctx.enter_context(nc.semaphore(name=f"kv{i}")) for i in range(len(pieces))]
        dve_sem = ctx.enter_context(nc.semaphore(name="dve_sem"))
        pe_sem = ctx.enter_context(nc.semaphore(name="pe_sem"))
        out_sem = ctx.enter_context(nc.semaphore(name="out_sem"))
        done_sem = ctx.enter_context(nc.semaphore(name="done_sem"))
        block = ctx.enter_context(nc.Block())

        @block.sync
        def _(sync):
            for pi, (a, b) in enumerate(SP_PIECES):
                sync.dma_start(kvt[:, a:b, :, :], kv_d[:, a:b, :, :]).then_inc(
                    kvsem[pi], 16
                )
            # bank-1 even strip as soon as its copy lands; the odd strip
            # follows on ACT's ring so the last DVE copy hides under this
            # DMA's trigger+stream.
            sync.wait_ge(dve_sem, 4)
            sync.dma_start(out_d[0:64, 256:512], out_big[0:64, 1, :, :]).then_inc(
                out_sem, 16
            )
            sync.wait_ge(out_sem, 48)
            sync.nop().then_inc(done_sem, 1)

        @block.gpsimd
        def _(gpsimd):
            # Semaphores persist across NEFF executions; clear them all at
            # the end (after every engine is provably done) so the kernel
            # is safe to run repeatedly.
            gpsimd.wait_ge(done_sem, 1)
            for s in [*kvsem, dve_sem, pe_sem, out_sem, done_sem]:
                gpsimd.sem_clear(s)

        @block.scalar
        def _(scalar):
            # Odd chunks stream on ACT's HWDGE ring, in parallel with SP.
            for pi, (a, b) in enumerate(ACT_PIECES):
                scalar.dma_start(kvt[:, a:b, :, :], kv_d[:, a:b, :, :]).then_inc(
                    kvsem[len(SP_PIECES) + pi], 16
                )
            # Then the bank-0 half of the output, while DVE still evacuates
            # bank 1.  ACT does no datapath work (an ACT copy feeding an ACT
            # dma_start raced -- the sequencer-level DMA trigger does not
            # order against the in-flight datapath op).
            scalar.wait_ge(dve_sem, 3)
            scalar.dma_start(out_d[:, 0:256], out_big[:, 0, :, :]).then_inc(
                out_sem, 16
            )
            scalar.wait_ge(dve_sem, 5)
            scalar.dma_start(
                out_d[64:128, 256:512], out_big[64:128, 1, :, :]
            ).then_inc(out_sem, 16)

        @block.vector
        def _(vector):
            vector.memset(acc[0][:], 0.0)
            vector.memset(acc[1][:], 0.0).then_inc(dve_sem, 1)
            # bank 0 strips while PE finishes bank 1 (different banks)
            vector.wait_ge(pe_sem, 16)
            vector.tensor_copy(out_big[0:64, 0, :, :], acc[0][0:64, :, 0, :]).then_inc(
                dve_sem, 1
            )
            vector.tensor_copy(
                out_big[64:128, 0, :, :], acc[0][64:128, :, 1, :]
            ).then_inc(dve_sem, 1)
            vector.wait_ge(pe_sem, 17)
            vector.tensor_copy(out_big[0:64, 1, :, :], acc[1][0:64, :, 0, :]).then_inc(
                dve_sem, 1
            )
            vector.tensor_copy(
                out_big[64:128, 1, :, :], acc[1][64:128, :, 1, :]
            ).then_inc(dve_sem, 1)

        @block.tensor
        def _(tensor):
            # PSUM accumulation is commutative, so chunks are processed
            # piece-by-piece in expected piece-ARRIVAL order (rings
            # alternating), draining each whole piece before the next wait.
            # Consuming in numeric chunk order ping-pongs between the two
            # rings every chunk and stalls on whichever ring is behind --
            # the rings were measured draining with up to 3us of skew.
            piece_order = [0, 4, 1, 5, 2, 6, 3, 7]
            processed = 0
            for pi in piece_order:
                a, b = pieces[pi]
                tensor.wait_ge(kvsem[pi], 16)
                if processed == 0:
                    tensor.wait_ge(dve_sem, 1)  # memsets done
                for s in range(a, b):
                    processed += 1
                    last = processed == CHUNKS
                    for q in range(NPAIR):
                        g, qq = divmod(q, 4)
                        mm = tensor.matmul(
                            acc[g][:, qq, :, :],
                            kvt[:, s, 0, q * 128 : (q + 1) * 128],
                            kvt[:, s, 1, q * 128 : (q + 1) * 128],
                            start=False,
                            stop=last and (q == 3 or q == NPAIR - 1),
                            skip_group_check=True,
                        )
                        if (last and q == 3) or q == NPAIR - 1:
                            mm.then_inc(pe_sem, 1)

    return nc


def _get_nc():
    global _nc_cache
    if _nc_cache is None:
        _nc_cache = _build_nc()
    return _nc_cache


def _make_in_maps(keys, values, write_strengths):
    # Fold sqrt(rho) into both operands in fp32, one fp8 cast for the full
    # arrays, then per-core: interleave chunk-permuted keys/values into the
    # (p, slot, 2, f) layout so each DMA partition line is contiguous.
    sq = np.sqrt(np.asarray(write_strengths, dtype=np.float32)).reshape(B * S, 1)
    kq = (keys.reshape(B * S, FD) * sq).astype(F8)
    vq = (values.reshape(B * S, FD) * sq).astype(F8)
    in_maps = []
    for c in range(N_CORES):
        sl = slice(c * NS, (c + 1) * NS)
        kc = kq[sl].reshape(CHUNKS, P, FD)
        vc = vq[sl].reshape(CHUNKS, P, FD)
        kv = np.stack([kc, vc], axis=1)[CHUNK_OF]      # (slot, 2, P, FD)
        in_maps.append(
            {"kv": np.ascontiguousarray(kv.transpose(2, 0, 1, 3))}
        )
    return in_maps


def _run(in_maps, **kwargs):
    from concourse.bass_utils import run_bass_kernel_spmd

    nc = _get_nc()
    return run_bass_kernel_spmd(nc, in_maps, core_ids=list(range(N_CORES)), **kwargs)


def _assemble(memory, results):
    parts = np.stack([r["delta"] for r in results], axis=0)  # (8, 128, 512)
    arr = parts.sum(axis=0, dtype=np.float64)  # (128, 512)
    blk = arr.reshape(128, 2, 4, Dv)
    delta_hkv = np.empty((H, Dk, Dv))
    for g in range(2):
        for q in range(4):
            delta_hkv[2 * (4 * g + q)] = blk[0:64, g, q, :]
            delta_hkv[2 * (4 * g + q) + 1] = blk[64:128, g, q, :]
    out = DECAY * np.asarray(memory, dtype=np.float64) + delta_hkv
    return out.astype(np.float32)


def kernel(memory, keys, values, write_strengths):
    memory = np.asarray(memory, dtype=np.float32)
    keys = np.asarray(keys, dtype=np.float32)
    values = np.asarray(values, dtype=np.float32)
    write_strengths = np.asarray(write_strengths, dtype=np.float32)

    in_maps = _make_in_maps(keys, values, write_strengths)
    res = _run(in_maps)
    return _assemble(memory, res.results)


if __name__ == "__main__":
    rng = np.random.default_rng(0)
    mem = rng.standard_normal((H, Dk, Dv), dtype=np.float32)
    k = rng.standard_normal((B, S, H, Dk), dtype=np.float32)
    v = rng.standard_normal((B, S, H, Dv), dtype=np.float32)
    w = rng.random((B, S), dtype=np.float32)
    out = kernel(mem, k, v, w)
    ref = DECAY * mem + np.einsum(
        "bs,bshk,bshv->hkv", w.astype(np.float64), k.astype(np.float64), v.astype(np.float64)
    )
    err = np.abs(out - ref).max() / np.abs(ref).max()
    print("self-check rel err:", err)
